# revision 37
# baseline (speedup 1.0000x reference)
"""Multi-head attention (B=2, Q=K=2048, H=16, D=V=64) on 8 Trainium2 cores.

Sharding: batch x heads. Core c handles batch b = c//4 and heads
[4*(c%4), 4*(c%4)+4) -- 4 (b,h) "pairs" per core, no cross-core comm.

Key optimization vs the naive version: the key_mask zeroes ~half the
keys, and masked keys contribute exactly 0 to both the softmax numerator
and denominator (exp*mask). So the host compacts K/V to the unmasked
keys only (padded to a multiple of 128; pad keys have V''=0 and a zero
denominator column, so they contribute exactly nothing). This halves
the score matrix and with it the TensorE and ActE work. The host also
pre-transposes and pre-casts Q/K/V'' to bf16, so the device does no
input conversion at all.

Device algorithm per (b,h) pair (flash-style, no max subtraction:
scores/8 ~ N(0,1), so exp() is far from fp32 overflow; the reference's
max subtraction cancels in the softmax ratio up to a vanishing
eps*exp(-max) term):

  for each q-block (512 wide):
    for each k-chunk (128 compacted keys):
      S^T[k,q] = (K-chunk d,k)^T @ (Q^T d,q)   on TensorE (bf16, fp32 acc)
      E = exp(S/8)                              PSUM -> SBUF bf16
      acc[0:65, q] += V''^T @ E                 on TensorE (V'' = [V*m | m])
    normalize in O^T orientation: den = acc[64] + eps on ScalarE (Copy
    with bias), rec = 1/den on VectorE (reciprocal_approx_fast),
    partition-broadcast rec on GpSimd, O^T = acc * rec on VectorE.
    The host untransposes.

The exp is split between engines to keep the ScalarE off the critical
path: the first two chunk-groups of each block use the ScalarE exp; the
last group is computed on the VectorE with a Schraudolph-style fast
exp2 — i16 = round(s*(2^7*log2(e)/8) + (16256 - 7.25)) bit-cast as
bfloat16 gives exp(s/8) to ~1.8% RMS on ~25% of the weights (~0.6% on
the final output, measured 0.9% total vs the 2% gate).

Software pipelining: QK matmuls for exp-group g are emitted before the
PV matmuls of group g-1, so the TensorE never waits on the exp engines.
The normalize runs on ScalarE/VectorE/GpSimd, so the TensorE stream is
matmuls only. PSUM: 2x3-bank score windows + 2 acc banks = 8. Input
DMAs are split across the SP and ActE queues; outputs stream out
per-block in bf16.
"""

import math
import sys

import numpy as np

sys.path.insert(0, "/opt/trn_rl_repo")

import ml_dtypes

import concourse.bacc as bacc
import concourse.mybir as mybir
import concourse.tile as tile
from concourse.bass_utils import run_bass_kernel_spmd

BF16NP = ml_dtypes.bfloat16

N_CORES = 8
B, Q, K, H, D, V = 2, 2048, 2048, 16, 64, 64
PAIRS = 4            # (b,h) pairs per core
QBW = 512            # q-block width
QB = Q // QBW        # 4 q-blocks
G = 3                # k-chunks per exp group (3 PSUM banks per window)
EPS = 1e-10

F32 = mybir.dt.float32
BF16 = mybir.dt.bfloat16
I16 = mybir.dt.int16

# Schraudolph fast-exp constants (bf16 bit pattern via int16):
# i16 = s * (2^7 * log2(e) / 8) + (2^7 * 127 - C);  C tuned for min RMS.
SCH_A = 1.4426950408889634 * 128.0 / 8.0
SCH_B = 16256.0 - 7.25

_cached = {}
LAST_RESULTS = None


def _build_program(kc):
    """kc = number of 128-key chunks after mask compaction."""
    nc = bacc.Bacc("TRN2", target_bir_lowering=False, debug=False, num_devices=N_CORES)

    qT = nc.dram_tensor("qT", [PAIRS, D, Q], BF16, kind="ExternalInput").ap()
    kT = nc.dram_tensor("kT", [PAIRS, D, kc * 128], BF16, kind="ExternalInput").ap()
    v = nc.dram_tensor("v", [PAIRS, 128, kc, V + 1], BF16, kind="ExternalInput").ap()
    # output: [pair, blk, V, q-in-block] (O^T; host untransposes)
    o = nc.dram_tensor("o", [PAIRS, QB, V, QBW], BF16, kind="ExternalOutput").ap()

    with tile.TileContext(nc) as tc:
        with (
            tc.sbuf_pool(name="persist", bufs=1) as persist,
            tc.sbuf_pool(name="epool", bufs=4) as epool,
            tc.sbuf_pool(name="norm", bufs=2) as normp,
            tc.sbuf_pool(name="osbp", bufs=3) as osbp,
            tc.psum_pool(name="win", bufs=2) as winp,
            tc.psum_pool(name="acc", bufs=2) as accp,
        ):
            # ---------------- input DMAs (no device-side conversion) -------
            # pairs 0/1 stream in on the SP queue, pairs 2/3 on the ActE
            # queue; K before Q before V'' so pair 0's first matmuls can
            # start as early as possible.
            qTb, kTb, vppb = [None] * PAIRS, [None] * PAIRS, [None] * PAIRS
            for p in range(PAIRS):
                kTb[p] = persist.tile(
                    [D, kc * 128], BF16, tag=f"kTb{p}", name=f"kTb{p}"
                )
                qTb[p] = persist.tile([D, Q], BF16, tag=f"qTb{p}", name=f"qTb{p}")
                vppb[p] = persist.tile(
                    [128, kc, V + 1], BF16, tag=f"vpp{p}", name=f"vpp{p}"
                )
            # pair 0's first chunks land first, split across both HWDGE
            # queues, so the first matmul can start ~4us earlier than a
            # whole-tile wait would allow
            kc_half = min(4 * 128, kc * 128)
            vc_head = min(3, kc)
            nc.sync.dma_start(out=kTb[0][:, 0:kc_half], in_=kT[0][:, 0:kc_half])
            nc.scalar.dma_start(out=qTb[0][:, 0:QBW], in_=qT[0][:, 0:QBW])
            nc.scalar.dma_start(
                out=vppb[0][:, 0:vc_head, :], in_=v[0][:, 0:vc_head, :]
            )
            if kc_half < kc * 128:
                nc.sync.dma_start(
                    out=kTb[0][:, kc_half : kc * 128],
                    in_=kT[0][:, kc_half : kc * 128],
                )
            if vc_head < kc:
                nc.sync.dma_start(
                    out=vppb[0][:, vc_head:kc, :], in_=v[0][:, vc_head:kc, :]
                )
            nc.sync.dma_start(out=qTb[0][:, QBW:Q], in_=qT[0][:, QBW:Q])
            # preload the ScalarE's Exp table while the boot DMAs stream
            warm_in = persist.tile([1, 1], F32, tag="warm_in")
            nc.vector.memset(warm_in, 0.0)
            warm_out = persist.tile([1, 1], F32, tag="warm_out")
            nc.scalar.activation(
                out=warm_out,
                in_=warm_in,
                func=mybir.ActivationFunctionType.Exp,
                scale=0.125,
            )
            for p in (1, 2, 3):
                eng = nc.scalar if p % 2 == 1 else nc.sync
                eng.dma_start(out=kTb[p], in_=kT[p])
                eng.dma_start(out=qTb[p], in_=qT[p])
                eng.dma_start(out=vppb[p], in_=v[p])

            # the last DVE_CHUNKS chunks run on the VectorE (fast exp), the
            # rest on the ScalarE in groups of 3, so per-block engine loads
            # balance under the TensorE's
            DVE_CHUNKS = min(2, kc)
            act_kc = kc - DVE_CHUNKS
            groups = [list(range(s, min(s + G, act_kc))) for s in range(0, act_kc, G)]
            groups.append(list(range(act_kc, kc)))

            def emit_mm2(p, acc, qbw, chunks, e):
                for i, c in enumerate(chunks):
                    nc.tensor.matmul(
                        acc[:, 0:qbw],
                        vppb[p][:, c, :],
                        e[:, i, 0:qbw],
                        start=(c == 0),
                        stop=(c == kc - 1),
                    )

            def emit_norm_head(acc, p, q0, qbw):
                """normalize front half of the previous block -- deps done.
                den lives in its own partition-0 SBUF tile written and read
                by the VectorE (the custom-DVE reciprocal misreads PSUM and
                offset-partition inputs); the numerator rows stream to SBUF
                via a ScalarE Copy so the acc PSUM bank frees two blocks
                before its next writer."""
                den = normp.tile([1, QBW], F32, tag="den")
                nc.vector.tensor_scalar_add(
                    out=den[:, 0:qbw], in0=acc[V : V + 1, 0:qbw], scalar1=EPS
                )
                # numerator copy on ScalarE: queued behind the next block's
                # exps it costs a ~0.9us acc-release stall per block, but on
                # the VectorE it serializes ahead of the fast-exp ops and
                # costs ~1.8us/block -- measured, ScalarE wins
                usb = normp.tile([V, QBW], F32, tag="usb")
                nc.scalar.activation(
                    out=usb[:, 0:qbw],
                    in_=acc[0:V, 0:qbw],
                    func=mybir.ActivationFunctionType.Copy,
                )
                rec = normp.tile([1, QBW], F32, tag="rec")
                nc.vector.reciprocal_approx_fast(out=rec[:, 0:qbw], in_=den[:, 0:qbw])
                bc = normp.tile([V, QBW], F32, tag="bc")
                nc.gpsimd.partition_broadcast(bc[:, 0:qbw], rec[:, 0:qbw])
                return usb, bc

            def emit_norm_tail(acc, p, q0, qbw, ub):
                """final multiply on VectorE, all-SBUF"""
                usb, bc = ub
                osb = osbp.tile([V, QBW], BF16, tag="osb")
                nc.vector.tensor_tensor(
                    out=osb[:, 0:qbw],
                    in0=usb[:, 0:qbw],
                    in1=bc[:, 0:qbw],
                    op=mybir.AluOpType.mult,
                )
                blk, off = q0 // QBW, q0 % QBW
                nc.sync.dma_start(
                    out=o[p, blk][:, off : off + qbw], in_=osb[:, 0:qbw]
                )

            # ---------------- main pipelined loops ----------------
            # norm(b-1) is emitted at the top of block b, so by the time each
            # of its ops reaches the head of its engine queue the deps are
            # long satisfied -- no engine blocks another through queue order.
            # Pipeline with a 2-group PV lag: mm2 for exp-group g runs ~2
            # groups of QK matmuls after g's scores land, covering the exp
            # engines' latency. The DVE group's PV matmuls and the block's
            # normalize slide into the NEXT block's instruction stream.
            # the final block runs as two half-width mini-blocks so its
            # serial normalize chain is half as long and the first half's
            # normalize overlaps the second half's matmuls
            mblocks = [(p, blk * QBW, QBW) for p in range(PAIRS) for blk in range(QB)]
            lp, lq0, _ = mblocks[-1]
            mblocks[-1:] = [(lp, lq0, QBW // 2), (lp, lq0 + QBW // 2, QBW // 2)]
            head_gi = min(1, len(groups) - 1)

            deferred = None  # previous mini-block awaiting PV tail + norm
            for p, q0, qbw in mblocks:
                acc = accp.tile([V + 1, QBW], F32, tag="acc")
                act_pending = []  # this block's ACT groups awaiting PV
                sch_work = []
                for gi, chunks in enumerate(groups):
                    last = gi == len(groups) - 1
                    win = winp.tile([128, G, QBW], F32, tag="win")
                    n = len(chunks)
                    e = epool.tile([128, G, QBW], BF16, tag="e")
                    for i, c in enumerate(chunks):
                        nc.tensor.matmul(
                            win[:, i, 0:qbw],
                            kTb[p][:, c * 128 : (c + 1) * 128],
                            qTb[p][:, q0 : q0 + qbw],
                            start=True,
                            stop=True,
                        )
                        if last:
                            sch_work.append((win, i, e))
                    if not last:
                        # exact exp on ScalarE
                        nc.scalar.activation(
                            out=e[:, :n, 0:qbw],
                            in_=win[:, :n, 0:qbw],
                            func=mybir.ActivationFunctionType.Exp,
                            scale=0.125,
                        )
                        act_pending.append((chunks, e))
                # this block's DVE fast-exp ops, queued ahead of the
                # previous block's den/rec so they never wait on them
                for win, i, e in sch_work:
                    nc.vector.tensor_scalar(
                        out=e[:, i, 0:qbw].bitcast(I16),
                        in0=win[:, i, 0:qbw],
                        scalar1=SCH_A,
                        scalar2=SCH_B,
                        op0=mybir.AluOpType.mult,
                        op1=mybir.AluOpType.add,
                    )
                # PE order: this block's first PV group FIRST, then the
                # previous block's deferred DVE-group PV (whose fast-exp
                # finished a full block ago), then the remaining PV groups.
                # This pushes each acc bank's first write ~a group later,
                # giving the previous tenant's release path extra slack.
                if act_pending:
                    emit_mm2(p, acc, qbw, *act_pending[0])
                if deferred is not None:
                    dacc, dp, dq0, dqbw, dpend = deferred
                    emit_mm2(dp, dacc, dqbw, *dpend)
                    bc = emit_norm_head(dacc, dp, dq0, dqbw)
                    deferred = (dacc, dp, dq0, dqbw, bc)
                for chunks, e in act_pending[1:]:
                    emit_mm2(p, acc, qbw, chunks, e)
                if deferred is not None:
                    dacc, dp, dq0, dqbw, bc = deferred
                    emit_norm_tail(dacc, dp, dq0, dqbw, bc)
                deferred = (acc, p, q0, qbw, (groups[-1], sch_work[0][2]))
            # flush the final mini-block
            dacc, dp, dq0, dqbw, dpend = deferred
            emit_mm2(dp, dacc, dqbw, *dpend)
            bc = emit_norm_head(dacc, dp, dq0, dqbw)
            emit_norm_tail(dacc, dp, dq0, dqbw, bc)

    nc.compile()
    return nc


def _get_program(kc):
    if kc not in _cached:
        _cached[kc] = _build_program(kc)
    return _cached[kc]


def _shard_inputs(queries, keys, values, key_mask):
    q = np.asarray(queries, dtype=np.float32)
    k = np.asarray(keys, dtype=np.float32)
    v = np.asarray(values, dtype=np.float32)
    m = np.asarray(key_mask)

    idx = [np.nonzero(m[b])[0] for b in range(B)]
    keff = max(len(ix) for ix in idx)
    kc = max(1, math.ceil(keff / 128))
    kp = kc * 128

    # [B, S, H, D] -> [B, H, D, S], bf16
    qT = np.ascontiguousarray(q.transpose(0, 2, 3, 1)).astype(BF16NP)

    # compacted K^T and V'' = [V*m | m], zero-padded to kp keys
    kT = np.zeros((B, H, D, kp), dtype=np.float32)
    vpp = np.zeros((B, H, kp, V + 1), dtype=np.float32)
    for b in range(B):
        ix = idx[b]
        n = len(ix)
        if n == 0:
            continue
        mb = m[b, ix].astype(np.float32)
        kT[b, :, :, :n] = k[b, ix].transpose(1, 2, 0)
        vpp[b, :, :n, :V] = (v[b, ix] * mb[:, None, None]).transpose(1, 0, 2)
        vpp[b, :, :n, V] = mb[None, :]
    kTb = kT.astype(BF16NP)
    # [B, H, kp, V+1] -> [B, H, 128(r), kc, V+1]  (key kk = c*128 + r)
    vppb = np.ascontiguousarray(
        vpp.reshape(B, H, kc, 128, V + 1).transpose(0, 1, 3, 2, 4)
    ).astype(BF16NP)

    in_maps = []
    for core in range(N_CORES):
        b, h0 = core // 4, (core % 4) * 4
        in_maps.append(
            {
                "qT": np.ascontiguousarray(qT[b, h0 : h0 + 4]),
                "kT": np.ascontiguousarray(kTb[b, h0 : h0 + 4]),
                "v": np.ascontiguousarray(vppb[b, h0 : h0 + 4]),
            }
        )
    return in_maps, kc


def kernel(queries, keys, values, key_mask):
    global LAST_RESULTS
    in_maps, kc = _shard_inputs(queries, keys, values, key_mask)
    nc = _get_program(kc)
    res = run_bass_kernel_spmd(nc, in_maps, list(range(N_CORES)))
    LAST_RESULTS = res

    out = np.empty((B, Q, H * V), dtype=np.float32)
    fully_masked = [not np.any(np.asarray(key_mask)[b]) for b in range(B)]
    for core in range(N_CORES):
        b, h0 = core // 4, (core % 4) * 4
        if fully_masked[b]:
            out[b] = 0.0
            continue
        # [PAIRS, QB, V, QBW] -> [PAIRS, Q, V]
        oc = (
            res.results[core]["o"]
            .astype(np.float32)
            .transpose(0, 1, 3, 2)
            .reshape(PAIRS, Q, V)
        )
        for p in range(PAIRS):
            h = h0 + p
            out[b, :, h * V : (h + 1) * V] = oc[p]
    return out
